# revision 9
# baseline (speedup 1.0000x reference)
"""Masked attention (B=16, S=1024, H=1024) on 8 TRN2 NeuronCores.

Strategy: pure data-parallel over batch — 2 batches per core, no collectives.

Sparsity: the mask zeroes ~half of the key columns per batch, and masked
columns contribute exactly-zero attention weights (exp(-1e9 + s) underflows
to 0 in f32, matching the reference bit-for-bit).  So the kernel gathers the
unmasked columns on the host, runs attention over a compact key/value axis of
UP columns (max unmasked count over batches, rounded up to 64), and the host
scatters the compact weight matrix back into the dense [S, S] output (masked
columns stay 0).  If any batch has zero unmasked columns the kernel falls
back to the dense path (UP = S, identity gather) through the same graph.

The QK product is reassociated to exploit the compact key axis: with
M = Wq^T Wk / sqrt(H), scores = X @ (M @ XU^T), so the per-batch projection
cost is H*H*UP instead of S*H*H — and the bias terms are rank-1: the
per-query term X@(Wq^T bk)/32 folds into T2 as a per-partition bias (evec),
the per-key term (bq Wk/32)@XU^T joins the mask bias mkb.

Scores are computed TRANSPOSED (scT[u, i], stationary = t2 tiles, moving =
xT): the softmax exp then lands directly in the e^T layout that the PV
matmul needs as its stationary operand, eliminating every PE transpose and
its psum->sbuf copy.  Scores are ~N(0,1) here, so exp needs no max
subtraction (f32 exp is exact-safe to |s|~80), and the row normalization
moves to the host: the kernel exports unnormalized e^T (which IS the compact
attention-weights output) and unnormalized e@V; the host divides both by
rowsum = sum_u e^T[u, i].  The mask bias is a per-partition (per-key) exp
bias, so the whole softmax is ONE scalar-engine op per score chunk.

Per batch (X = input[b] [S, H], XU = unmasked-gathered columns [UP, H]):
  t2  = M @ XU^T + evec x 1  -> [H, UP]   (ACT cast adds evec per partition)
  vU  = XU @ Wv.T + bv       -> [UP, H]
  eT  = exp(t2^T-matmul + mkb[u])         -> [UP, S]  (attwc output, unnorm)
  att = eT.T @ vU            -> [S, H]    (unnormalized; host divides)

All TensorEngine operands are bf16 (pre-cast on host), accumulation f32 in
PSUM.  End-to-end rel err vs the f32 reference ~5e-3.
"""
import numpy as np
import ml_dtypes

import concourse.bass as bass
import concourse.mybir as mybir
from concourse import bacc
from concourse.tile import TileContext
from concourse.bass_utils import run_bass_kernel_spmd

B, S, H = 16, 1024, 1024
P = 128
NCORES = 8
B_LOC = B // NCORES          # batches per core
KT = H // P                  # 8 contraction tiles
RT = S // P                  # 8 query row blocks
NFREE = 512                  # matmul moving free dim (one PSUM bank)
BF16 = mybir.dt.bfloat16
F32 = mybir.dt.float32

_BUILD_CACHE = {}


def _chunks(total, step=NFREE):
    out = []
    o = 0
    while o < total:
        out.append((o, min(step, total - o)))
        o += step
    return out


def build(UP):
    """Build the SPMD graph for a compact key axis of UP columns."""
    if UP in _BUILD_CACHE:
        return _BUILD_CACHE[UP]
    assert UP % 64 == 0
    # u blocks of <=128: stationary tiles (scT) / contraction blocks (PV, vU)
    UBL = _chunks(UP, P)
    NUT = len(UBL)
    # balanced u chunks of <=NFREE for T2's moving side: chunks >=256 cols
    # keep the per-matmul LDWEIGHTS hidden behind the previous chain
    if UP <= NFREE:
        UCH = [(0, UP)]
    else:
        c0 = ((UP + 1) // 2 + 31) // 32 * 32
        UCH = [(0, c0), (c0, UP - c0)]
    SCH = _chunks(S)
    HCH = _chunks(H)

    nc = bacc.Bacc()

    # All inputs arrive pre-tiled in SBUF layout (partition-major, contiguous
    # per partition) so DMA bursts are kilobytes, not 256B strided runs.
    xT = nc.declare_dram_parameter("xT", [B_LOC, P, KT, S], BF16, isOutput=False)
    mp = nc.declare_dram_parameter("mp", [KT, P, KT, P], BF16, isOutput=False)
    wvp = nc.declare_dram_parameter("wvp", [len(HCH), P, KT, NFREE], BF16, isOutput=False)
    evp = nc.declare_dram_parameter("evp", [P, KT], F32, isOutput=False)
    bvr = nc.declare_dram_parameter("bvr", [P, H], BF16, isOutput=False)
    mkb = nc.declare_dram_parameter("mkb", [B_LOC, P, NUT], F32, isOutput=False)
    att = nc.declare_dram_parameter("att", [B_LOC, S, H], BF16, isOutput=True)
    attwc = nc.declare_dram_parameter("attwc", [B_LOC, NUT, P, S], BF16, isOutput=True)

    with TileContext(nc) as tc:
        with (
            tc.tile_pool(name="const", bufs=1) as constp,
            tc.tile_pool(name="wpool", bufs=1) as wpool,
            tc.tile_pool(name="xpool", bufs=2) as xpool,
            tc.tile_pool(name="qkv", bufs=1) as qkvp,
            tc.tile_pool(name="soft", bufs=3) as soft,
            tc.tile_pool(name="psmm", bufs=6, space="PSUM") as psmm,
        ):
            ev_t = constp.tile([P, KT], F32)
            bv_t = constp.tile([P, H], BF16)

            # DMA issue is serialized per engine, so split streams: the m
            # blocks (consumed ot-by-ot by the T2 chains, first) issue on the
            # scalar engine's HWDGE concurrently with xT chunks on sync's.
            # The first T2 chain only needs m[0] + xT0[:, :, 0:UP], so those
            # lead both queues; wv interleaves mid-stream for vU.
            m_t = wpool.tile([P, KT, KT, P], BF16)      # [p, o-block, k-tile, o']
            wv_t = wpool.tile([P, len(HCH), KT, NFREE], BF16)  # [p, h-chunk, k-tile, o']
            xT0_t = xpool.tile([P, KT, S], BF16, name="xT0_t", tag="xT")

            nc.scalar.dma_start(out=ev_t, in_=evp[:, :])
            for ot in range(5):
                nc.scalar.dma_start(out=m_t[:, ot], in_=mp[ot])
            nc.scalar.dma_start(out=wv_t[:, 0], in_=wvp[0])
            for ot in range(5, KT):
                nc.scalar.dma_start(out=m_t[:, ot], in_=mp[ot])
            nc.scalar.dma_start(out=wv_t[:, 1], in_=wvp[1])
            for off, csz in UCH:
                nc.sync.dma_start(out=xT0_t[:, :, off:off + csz],
                                  in_=xT[0][:, :, off:off + csz])
            if UP < S:
                nc.sync.dma_start(out=xT0_t[:, :, UP:S], in_=xT[0][:, :, UP:S])
            nc.sync.dma_start(out=bv_t, in_=bvr[:, :])

            def load_batch_inputs(b):
                # unmasked tokens are a host-permuted prefix of xT, so the
                # compact key/value view is just a slice of the same tile
                if b == 0:
                    xb_t = xT0_t
                else:
                    xb_t = xpool.tile([P, KT, S], BF16, name="xT_t", tag="xT")
                    nc.sync.dma_start(out=xb_t, in_=xT[b])
                mkb_t = xpool.tile([P, NUT], F32, name="mkb_t", tag="mkb")
                nc.sync.dma_start(out=mkb_t, in_=mkb[b])
                return xb_t, mkb_t

            next_inputs = load_batch_inputs(0)
            for b in range(B_LOC):
                xT_t, mkb_t = next_inputs

                t2_t = qkvp.tile([P, KT, UP], BF16, name="t2_t", tag="t2", bufs=2)
                v_t = qkvp.tile([P, NUT, H], BF16, name="v_t", tag="v", bufs=2)
                eT_t = qkvp.tile([P, NUT, S], BF16, name="eT_t", tag="eT", bufs=2)

                # ---- t2[h, u] = sum_h' M[h, h'] XU[u, h'] + evec[h]
                # (stationary = M^T tiles, moving = the compact prefix of xT;
                # the psum->sbuf cast on ACT adds the per-partition evec) ----
                for ot in range(KT):
                    for off, csz in UCH:
                        sl = slice(off, off + csz)
                        ps_q = psmm.tile([P, NFREE], F32, name="ps_q", tag="mm")[:, :csz]
                        for kt in range(KT):
                            nc.tensor.matmul(ps_q, m_t[:, ot, kt],
                                             xT_t[:, kt, sl], start=(kt == 0), stop=(kt == KT - 1))
                        nc.scalar.activation(out=t2_t[:, ot, sl], in_=ps_q,
                                             func=mybir.ActivationFunctionType.Identity,
                                             bias=ev_t[:, ot:ot + 1])

                def emit_scT(ci):
                    # scT[u, i] = sum_h t2[h, u] X[i, h] for i-chunk ci; the
                    # softmax is ONE ACT op: eT = exp(scT + mkb[u]) straight
                    # from PSUM, with the mask/pad/key-bias as exp bias.
                    off, csz = SCH[ci]
                    sl = slice(off, off + csz)
                    for ui, (uo, usz) in enumerate(UBL):
                        ps_s = psmm.tile([P, NFREE], F32, name="ps_s", tag="mm")[:usz, :csz]
                        for kt in range(KT):
                            nc.tensor.matmul(ps_s, t2_t[:, kt, uo:uo + usz],
                                             xT_t[:, kt, sl], start=(kt == 0), stop=(kt == KT - 1))
                        nc.scalar.activation(out=eT_t[:usz, ui, sl], in_=ps_s,
                                             func=mybir.ActivationFunctionType.Exp,
                                             bias=mkb_t[:usz, ui:ui + 1], scale=1.0)

                def emit_v():
                    # vU[u, o] = sum_h XU[u, h] Wv[o, h] + bv[o]
                    for ci, (off, csz) in enumerate(HCH):
                        sl = slice(off, off + csz)
                        for ui, (uo, usz) in enumerate(UBL):
                            ps_v = psmm.tile([P, NFREE], F32, name="ps_v", tag="mm")[:usz, :csz]
                            for kt in range(KT):
                                nc.tensor.matmul(ps_v, xT_t[:, kt, uo:uo + usz],
                                                 wv_t[:, ci, kt, 0:csz], start=(kt == 0), stop=(kt == KT - 1))
                            nc.vector.tensor_tensor(out=v_t[:usz, ui, sl], in0=ps_v,
                                                    in1=bv_t[:usz, sl], op=mybir.AluOpType.add)

                def emit_pv(r):
                    # att[i, h] = sum_u e[i, u] v[u, h], unnormalized; eT is
                    # already the stationary layout.  Cast rides the DVE.
                    at_t = soft.tile([P, H], BF16, name="at_t", tag="at")
                    for off, csz in HCH:
                        sl = slice(off, off + csz)
                        ps_a = psmm.tile([P, NFREE], F32, name="ps_a", tag="mm")[:, :csz]
                        for ui, (uo, usz) in enumerate(UBL):
                            nc.tensor.matmul(ps_a, eT_t[:usz, ui, r * P:(r + 1) * P],
                                             v_t[:usz, ui, sl],
                                             start=(ui == 0), stop=(ui == NUT - 1))
                        nc.vector.tensor_copy(out=at_t[:, sl], in_=ps_a)
                    nc.sync.dma_start(out=att[b, r * P:(r + 1) * P, :], in_=at_t)

                # Emission order: every consumer sits >=1 full PE phase after
                # its producer, so the only cross-engine latency on the PE
                # critical path is the ACT exp of the last score chunk.
                emit_scT(0)
                emit_v()
                # Prefetch next batch's inputs now, so their sync-queue DMAs
                # sit ahead of this batch's output DMAs in the engine stream.
                if b + 1 < B_LOC:
                    next_inputs = load_batch_inputs(b + 1)
                emit_scT(1)
                # compact unnormalized weights out (= eT), one DMA per u-block
                for ui, (uo, usz) in enumerate(UBL):
                    nc.sync.dma_start(out=attwc[b, ui, 0:usz, :], in_=eT_t[:usz, ui, :])
                for r in range(RT):
                    emit_pv(r)

    nc.finalize()
    _BUILD_CACHE[UP] = nc
    return nc


def _bf16(x):
    return np.ascontiguousarray(x.astype(ml_dtypes.bfloat16))


def kernel(input, mask, Wq, bq, Wk, bk, Wv, bv):
    input = np.asarray(input, dtype=np.float32)
    mask = np.asarray(mask)
    scale = np.float32(1.0 / np.sqrt(H))

    # Fused scores: scores = X @ (M @ XU^T) with M = Wq^T Wk / sqrt(H); the
    # bias cross-terms are rank-1: evec (per query, folded into t2's cast
    # bias), w1vec@XU^T + dconst (per key, folded into the exp bias mkb).
    Wq = np.asarray(Wq, dtype=np.float32)
    Wk = np.asarray(Wk, dtype=np.float32)
    bq = np.asarray(bq, dtype=np.float32)
    bk = np.asarray(bk, dtype=np.float32)
    MT = (Wk.T @ Wq) * scale
    w1vec = (bq * scale) @ Wk
    evec = (bk @ Wq) * scale
    dconst = np.float32((bq * scale) @ bk)
    # Pre-tile weights: per-output-block, partition-major [blk, p, t, inner].
    mp = np.ascontiguousarray(
        _bf16(MT).reshape(KT, P, KT, P).transpose(2, 1, 0, 3))
    wvp = np.ascontiguousarray(
        _bf16(np.asarray(Wv).T).reshape(KT, P, H // NFREE, NFREE).transpose(2, 1, 0, 3))
    evp = np.ascontiguousarray(evec.reshape(KT, P).T.astype(np.float32))
    bvr = _bf16(np.broadcast_to(np.asarray(bv, dtype=np.float32), (P, H)))

    # Permute each batch's token axis so unmasked tokens form a prefix: the
    # compact key/value block is then a slice of the (permuted) xT tile and
    # needs no separate transfer.  Queries are order-independent; outputs are
    # un-permuted below.
    m = np.asarray(mask[:, 0, 0, :])                     # [B, S]
    idxs = [np.nonzero(m[b] != 0)[0] for b in range(B)]
    ucounts = [len(ix) for ix in idxs]
    sparse = min(ucounts) > 0 and max(ucounts) < S
    if sparse:
        UP = max(P, ((max(ucounts) + 63) // 64) * 64)
        perms = [np.concatenate([idxs[b], np.nonzero(m[b] == 0)[0]]) for b in range(B)]
    else:
        UP = S
        idxs = [np.arange(S) for _ in range(B)]
        ucounts = [S] * B
        perms = [np.arange(S) for _ in range(B)]
    UBL = _chunks(UP, P)
    NUT = len(UBL)

    in_maps = []
    for c in range(NCORES):
        xb = np.stack([input[c * B_LOC + bl][perms[c * B_LOC + bl]]
                       for bl in range(B_LOC)])          # [B_LOC, S, H] permuted rows
        xTf = _bf16(xb.transpose(0, 2, 1))               # [B_LOC, H, S]
        mkb = np.full((B_LOC, P, NUT), -1e9, dtype=np.float32)
        for bl in range(B_LOC):
            gb = c * B_LOC + bl
            row = np.where(m[gb][perms[gb]][:UP] == 0, np.float32(-1e9), np.float32(0.0))
            row = row + xb[bl, :UP].astype(np.float32) @ w1vec + dconst
            mkb[bl] = np.pad(row, (0, NUT * P - UP),
                             constant_values=np.float32(-1e9)).reshape(NUT, P).T
        xT_t = np.ascontiguousarray(
            xTf.reshape(B_LOC, KT, P, S).transpose(0, 2, 1, 3))
        in_maps.append({
            "xT": xT_t,
            "mp": mp, "wvp": wvp,
            "evp": evp, "bvr": bvr, "mkb": mkb,
        })

    nc = build(UP)
    res = run_bass_kernel_spmd(nc, in_maps, core_ids=list(range(NCORES)))
    att = np.empty((B, S, H), dtype=np.float32)
    attw = np.zeros((B, S, S), dtype=np.float32)
    for c in range(NCORES):
        att_c = res.results[c]["att"]                    # [B_LOC, S, H] bf16, unnormalized
        awc = res.results[c]["attwc"]                    # [B_LOC, NUT, P, S] bf16 eT, padded
        for bl in range(B_LOC):
            gb = c * B_LOC + bl
            eT = np.concatenate([awc[bl, ui, :usz] for ui, (uo, usz) in enumerate(UBL)])
            eT = eT[:ucounts[gb]].astype(np.float32)     # [uc, S] valid unmasked rows
            rowsum = eT.sum(0, dtype=np.float64)         # [S] softmax denominator
            if np.any(rowsum == 0.0):                    # all-masked batch: softmax
                rowsum = np.ones_like(rowsum)            # over -1e9 is uniform
                eT = np.full_like(eT, 1.0 / S)
                xf = input[gb].astype(np.float32)
                v = xf @ np.asarray(Wv, dtype=np.float32).T + np.asarray(bv, np.float32)
                att[gb] = v.mean(0)
                attw[gb] = 1.0 / S
                continue
            inv = (1.0 / rowsum).astype(np.float32)
            att[gb][perms[gb]] = att_c[bl].astype(np.float32) * inv[:, None]
            tmp = np.zeros((S, S), dtype=np.float32)
            tmp[:, idxs[gb]] = (eT * inv[None, :]).T
            attw[gb][perms[gb]] = tmp
    return att, attw


# revision 10
# speedup vs baseline: 1.1138x; 1.1138x over previous
"""Masked attention (B=16, S=1024, H=1024) on 8 TRN2 NeuronCores.

Strategy: pure data-parallel over batch — 2 batches per core, no collectives.

Sparsity: the mask zeroes ~half of the key columns per batch, and masked
columns contribute exactly-zero attention weights (exp(-1e9 - max) underflows
to 0 in f32, matching the reference bit-for-bit).  So the kernel gathers the
unmasked columns on the host, runs attention over a compact key/value axis of
UP columns (max unmasked count over batches, rounded up to 64), and the host
scatters the compact weight matrix back into the dense [S, S] output (masked
columns stay 0).  If any batch has zero unmasked columns the kernel falls
back to the dense path (UP = S, identity gather) through the same graph.

The QK product is reassociated to exploit the compact key axis: with
M = Wq^T Wk / sqrt(H), scores = X @ (M @ XU^T), so the per-batch projection
cost is H*H*UP instead of S*H*H (T1 = X@M) — UP < S makes the right
association strictly cheaper and the K projection still disappears.

Per batch (X = input[b] [S, H], XU = unmasked-gathered columns [UP, H]):
  t2  = M @ XU^T (+ bias cross-terms folded into mkb/d)  -> [H, UP]
  vU  = XU @ Wv.T + bv                                   -> [UP, H]
  s   = X @ t2 + d[i] + mkb[u]   -> [S, UP]  (mkb = -1e9 on masked/pad cols)
  e   = exp(s - rowmax); attwc = e / rowsum (compact weights out)
  att = (eT.T @ vU) / rowsum     -> e transposed on the PE

All TensorEngine operands are bf16 (pre-cast on host), accumulation f32 in
PSUM, softmax statistics f32.  End-to-end rel err vs the f32 reference ~5e-3.

NOTE a denser schedule (scores computed transposed, exp straight from PSUM,
no PE transposes) was tried and is ~19% SLOWER end-to-end: the sustained
back-to-back 512-col matmul stream trips the power throttle and drops the
PE clock by ~20%.  The transposes and per-iteration softmax bubbles in this
schedule act as duty-cycling that keeps the PE at full clock.
"""
import numpy as np
import ml_dtypes

import concourse.bass as bass
import concourse.mybir as mybir
from concourse import bacc
from concourse.tile import TileContext
from concourse.bass_utils import run_bass_kernel_spmd
from concourse.masks import make_identity

B, S, H = 16, 1024, 1024
P = 128
NCORES = 8
B_LOC = B // NCORES          # batches per core
KT = H // P                  # 8 contraction tiles
RT = S // P                  # 8 query row blocks
NFREE = 512                  # matmul moving free dim (one PSUM bank)
DEPTH = 4                    # softmax->PV software pipeline depth (row blocks)
BF16 = mybir.dt.bfloat16
F32 = mybir.dt.float32

_BUILD_CACHE = {}


def _chunks(total, step=NFREE):
    out = []
    o = 0
    while o < total:
        out.append((o, min(step, total - o)))
        o += step
    return out


def build(UP):
    """Build the SPMD graph for a compact key axis of UP columns."""
    if UP in _BUILD_CACHE:
        return _BUILD_CACHE[UP]
    assert UP % 64 == 0
    # u blocks of <=128 for the stationary/contraction side (vU, transposes, PV)
    UBL = _chunks(UP, P)
    NUT = len(UBL)
    # balanced u chunks of <=NFREE for the moving side (T2, scores): keeping
    # the chunks >=256 cols hides the per-matmul LDWEIGHTS behind the chain
    if UP <= NFREE:
        UCH = [(0, UP)]
    else:
        c0 = ((UP + 1) // 2 + 31) // 32 * 32
        UCH = [(0, c0), (c0, UP - c0)]
    HCH = _chunks(H)

    nc = bacc.Bacc()

    # All inputs arrive pre-tiled in SBUF layout (partition-major, contiguous
    # per partition) so DMA bursts are kilobytes, not 256B strided runs.
    xT = nc.declare_dram_parameter("xT", [B_LOC, P, KT, S], BF16, isOutput=False)
    mp = nc.declare_dram_parameter("mp", [KT, P, KT, P], BF16, isOutput=False)
    wvp = nc.declare_dram_parameter("wvp", [len(HCH), P, KT, NFREE], BF16, isOutput=False)
    bvr = nc.declare_dram_parameter("bvr", [P, H], BF16, isOutput=False)
    dp = nc.declare_dram_parameter("dp", [B_LOC, P, RT], F32, isOutput=False)
    mkb = nc.declare_dram_parameter("mkb", [B_LOC, P, UP], BF16, isOutput=False)
    att = nc.declare_dram_parameter("att", [B_LOC, S, H], BF16, isOutput=True)
    attwc = nc.declare_dram_parameter("attwc", [B_LOC, S, UP], BF16, isOutput=True)

    with TileContext(nc) as tc:
        with (
            tc.tile_pool(name="const", bufs=1) as constp,
            tc.tile_pool(name="wpool", bufs=1) as wpool,
            tc.tile_pool(name="xpool", bufs=2) as xpool,
            tc.tile_pool(name="qkv", bufs=1) as qkvp,
            tc.tile_pool(name="soft", bufs=DEPTH + 1) as soft,
            tc.tile_pool(name="stats", bufs=DEPTH + 2) as stats,
            tc.tile_pool(name="psmm", bufs=6, space="PSUM") as psmm,
            tc.tile_pool(name="pstr", bufs=2, space="PSUM") as pstr,
        ):
            ident = constp.tile([P, P], BF16)
            make_identity(nc, ident)
            bv_t = constp.tile([P, H], BF16)

            # DMA issue is serialized per engine, so split streams: the m
            # blocks (consumed ot-by-ot by the T2 chains, first) issue on the
            # scalar engine's HWDGE concurrently with xT chunks on sync's.
            # m must fully precede wv: the T2 chains consume m at ~2.8 GB/s
            # effective, any wv transfer in between stalls the T2 tail.
            m_t = wpool.tile([P, KT, KT, P], BF16)      # [p, o-block, k-tile, o']
            wv_t = wpool.tile([P, len(HCH), KT, NFREE], BF16)  # [p, h-chunk, k-tile, o']
            xT0_t = xpool.tile([P, KT, S], BF16, name="xT0_t", tag="xT")

            for ot in range(KT):
                nc.scalar.dma_start(out=m_t[:, ot], in_=mp[ot])
            for ci in range(len(HCH)):
                nc.scalar.dma_start(out=wv_t[:, ci], in_=wvp[ci])
            for off, csz in UCH:
                nc.sync.dma_start(out=xT0_t[:, :, off:off + csz],
                                  in_=xT[0][:, :, off:off + csz])
            if UP < S:
                nc.sync.dma_start(out=xT0_t[:, :, UP:S], in_=xT[0][:, :, UP:S])
            nc.sync.dma_start(out=bv_t, in_=bvr[:, :])

            def load_batch_inputs(b):
                # unmasked tokens are a host-permuted prefix of xT, so the
                # compact key/value view is just a slice of the same tile
                if b == 0:
                    xb_t = xT0_t
                else:
                    xb_t = xpool.tile([P, KT, S], BF16, name="xT_t", tag="xT")
                    nc.sync.dma_start(out=xb_t, in_=xT[b])
                mkb_t = xpool.tile([P, UP], BF16, name="mkb_t", tag="mkb")
                nc.sync.dma_start(out=mkb_t, in_=mkb[b])
                d_t = xpool.tile([P, RT], F32, name="d_t", tag="d")
                nc.sync.dma_start(out=d_t, in_=dp[b])
                return xb_t, mkb_t, d_t

            next_inputs = load_batch_inputs(0)
            for b in range(B_LOC):
                xT_t, mkb_t, d_t = next_inputs

                t2_t = qkvp.tile([P, KT, UP], BF16, name="t2_t", tag="t2", bufs=2)
                v_t = qkvp.tile([P, NUT, H], BF16, name="v_t", tag="v", bufs=2)

                # ---- t2[h, u] = sum_h' M[h, h'] XU[u, h'] (stationary = M^T
                # tiles, moving = the compact prefix of xT); psum->sbuf casts
                # ride ACT, which is idle during the projection phase ----
                for ot in range(KT):
                    for off, csz in UCH:
                        sl = slice(off, off + csz)
                        ps_q = psmm.tile([P, NFREE], F32, name="ps_q", tag="mm")[:, :csz]
                        for kt in range(KT):
                            nc.tensor.matmul(ps_q, m_t[:, ot, kt],
                                             xT_t[:, kt, sl], start=(kt == 0), stop=(kt == KT - 1))
                        nc.scalar.activation(out=t2_t[:, ot, sl], in_=ps_q,
                                             func=mybir.ActivationFunctionType.Copy)

                def emit_v():
                    # vU[u, o] = sum_h XU[u, h] Wv[o, h] + bv[o]
                    for ci, (off, csz) in enumerate(HCH):
                        sl = slice(off, off + csz)
                        for ui, (uo, usz) in enumerate(UBL):
                            ps_v = psmm.tile([P, NFREE], F32, name="ps_v", tag="mm")[:usz, :csz]
                            for kt in range(KT):
                                nc.tensor.matmul(ps_v, xT_t[:, kt, uo:uo + usz],
                                                 wv_t[:, ci, kt, 0:csz], start=(kt == 0), stop=(kt == KT - 1))
                            nc.vector.tensor_tensor(out=v_t[:usz, ui, sl], in0=ps_v,
                                                    in1=bv_t[:usz, sl], op=mybir.AluOpType.add)

                # ---- attention, software-pipelined over row blocks ----
                def emit_scores(r):
                    sc_t = soft.tile([P, UP], F32, name="sc_t", tag="sc")
                    for off, csz in UCH:
                        sl = slice(off, off + csz)
                        ps_s = psmm.tile([P, NFREE], F32, name="ps_s", tag="mm")[:, :csz]
                        for kt in range(KT):
                            nc.tensor.matmul(ps_s, xT_t[:, kt, r * P:(r + 1) * P],
                                             t2_t[:, kt, sl], start=(kt == 0), stop=(kt == KT - 1))
                        nc.vector.scalar_tensor_tensor(
                            out=sc_t[:, sl], in0=ps_s, scalar=d_t[:, r:r + 1],
                            in1=mkb_t[:, sl], op0=mybir.AluOpType.add, op1=mybir.AluOpType.add)
                    return sc_t

                def emit_softmax(r, sc_t):
                    negmax = stats.tile([P, 1], F32, name="negmax", tag="negmax")
                    nc.vector.reduce_max(out=negmax, in_=sc_t, axis=mybir.AxisListType.X, negate=True)
                    e_t = soft.tile([P, UP], BF16, name="e_t", tag="e")
                    rowsum = stats.tile([P, 1], F32, name="rowsum", tag="rowsum")
                    nc.scalar.activation(out=e_t, in_=sc_t, func=mybir.ActivationFunctionType.Exp,
                                         bias=negmax, scale=1.0, accum_out=rowsum)
                    recip = stats.tile([P, 1], F32, name="recip", tag="recip")
                    nc.vector.reciprocal(out=recip, in_=rowsum)

                    # compact attention-weights output: attwc = e * recip
                    p_t = soft.tile([P, UP], BF16, name="p_t", tag="p")
                    nc.vector.tensor_scalar_mul(p_t, e_t, recip)
                    nc.sync.dma_start(out=attwc[b, r * P:(r + 1) * P, :], in_=p_t)
                    return e_t, recip

                def emit_pv(r, e_t, recip):
                    # transpose e on the PE: eT[u, i] tiles (gpsimd cannot
                    # read PSUM, so the psum->sbuf copies stay on ACT)
                    eT_t = soft.tile([P, NUT, P], BF16, name="eT_t", tag="eT")
                    for ui, (uo, usz) in enumerate(UBL):
                        ps_t = pstr.tile([P, P], BF16, name="ps_t", tag="tr")[:usz]
                        nc.tensor.transpose(ps_t, e_t[:, uo:uo + usz], ident)
                        nc.scalar.activation(out=eT_t[:usz, ui], in_=ps_t,
                                             func=mybir.ActivationFunctionType.Copy)

                    # att[i, h] = sum_u e[i, u] v[u, h], normalized by recip
                    at_t = soft.tile([P, H], BF16, name="at_t", tag="at")
                    for off, csz in HCH:
                        sl = slice(off, off + csz)
                        ps_a = psmm.tile([P, NFREE], F32, name="ps_a", tag="mm")[:, :csz]
                        for ui, (uo, usz) in enumerate(UBL):
                            nc.tensor.matmul(ps_a, eT_t[:usz, ui], v_t[:usz, ui, sl],
                                             start=(ui == 0), stop=(ui == NUT - 1))
                        nc.vector.tensor_scalar_mul(at_t[:, sl], ps_a, recip)
                    nc.sync.dma_start(out=att[b, r * P:(r + 1) * P, :], in_=at_t)

                # Emission order: sc(r), pv(r-DEPTH), sm(r) — the PE sees
                # DEPTH score blocks of lookahead, so the batch tail (exp ->
                # transpose-copy -> PV of the last blocks) overlaps the
                # still-queued PV matmuls of earlier blocks instead of
                # draining serially after the last score block.  vU sits
                # after sm(1): wv streams in behind the m blocks while the
                # PE chews T2 + the first two score blocks.
                args = {}
                sc = {}
                sc[0] = emit_scores(0)
                args[0] = emit_softmax(0, sc[0])
                sc[1] = emit_scores(1)
                args[1] = emit_softmax(1, sc[1])
                emit_v()
                # Prefetch next batch's inputs now, so their sync-queue DMAs
                # sit ahead of this batch's output DMAs in the engine stream.
                if b + 1 < B_LOC:
                    next_inputs = load_batch_inputs(b + 1)
                for r in range(2, DEPTH):
                    sc[r] = emit_scores(r)
                    args[r] = emit_softmax(r, sc[r])
                for r in range(DEPTH, RT):
                    sc[r] = emit_scores(r)
                    emit_pv(r - DEPTH, *args[r - DEPTH])
                    args[r] = emit_softmax(r, sc[r])
                for r in range(RT - DEPTH, RT):
                    emit_pv(r, *args[r])

    nc.finalize()
    _BUILD_CACHE[UP] = nc
    return nc


def _bf16(x):
    return np.ascontiguousarray(x.astype(ml_dtypes.bfloat16))


def kernel(input, mask, Wq, bq, Wk, bk, Wv, bv):
    input = np.asarray(input, dtype=np.float32)
    mask = np.asarray(mask)
    scale = np.float32(1.0 / np.sqrt(H))

    # Fused scores: scores = X @ (M @ XU^T) with M = Wq^T Wk / sqrt(H); the
    # bias cross-terms are rank-1: w1vec@XU^T folds into mkb (per key column),
    # evec/dconst feed the host-computed per-row term d.
    Wq = np.asarray(Wq, dtype=np.float32)
    Wk = np.asarray(Wk, dtype=np.float32)
    bq = np.asarray(bq, dtype=np.float32)
    bk = np.asarray(bk, dtype=np.float32)
    MT = (Wk.T @ Wq) * scale
    w1vec = (bq * scale) @ Wk
    evec = (bk @ Wq) * scale
    dconst = np.float32((bq * scale) @ bk)
    # Pre-tile weights: per-output-block, partition-major [blk, p, t, inner].
    mp = np.ascontiguousarray(
        _bf16(MT).reshape(KT, P, KT, P).transpose(2, 1, 0, 3))
    wvp = np.ascontiguousarray(
        _bf16(np.asarray(Wv).T).reshape(KT, P, H // NFREE, NFREE).transpose(2, 1, 0, 3))
    bvr = _bf16(np.broadcast_to(np.asarray(bv, dtype=np.float32), (P, H)))

    # Permute each batch's token axis so unmasked tokens form a prefix: the
    # compact key/value block is then a slice of the (permuted) xT tile and
    # needs no separate transfer.  Queries are order-independent; outputs are
    # un-permuted below.
    m = np.asarray(mask[:, 0, 0, :])                     # [B, S]
    idxs = [np.nonzero(m[b] != 0)[0] for b in range(B)]
    ucounts = [len(ix) for ix in idxs]
    sparse = min(ucounts) > 0 and max(ucounts) < S
    if sparse:
        UP = max(P, ((max(ucounts) + 63) // 64) * 64)
        perms = [np.concatenate([idxs[b], np.nonzero(m[b] == 0)[0]]) for b in range(B)]
    else:
        UP = S
        idxs = [np.arange(S) for _ in range(B)]
        ucounts = [S] * B
        perms = [np.arange(S) for _ in range(B)]

    in_maps = []
    for c in range(NCORES):
        xb = np.stack([input[c * B_LOC + bl][perms[c * B_LOC + bl]]
                       for bl in range(B_LOC)])          # [B_LOC, S, H] permuted rows
        xTf = _bf16(xb.transpose(0, 2, 1))               # [B_LOC, H, S]
        mkb = np.zeros((B_LOC, P, UP), dtype=ml_dtypes.bfloat16)
        d = (xb.astype(np.float32) @ evec + dconst).astype(np.float32)   # [B_LOC, S]
        dp = np.ascontiguousarray(d.reshape(B_LOC, RT, P).transpose(0, 2, 1))
        for bl in range(B_LOC):
            gb = c * B_LOC + bl
            row = np.where(m[gb][perms[gb]][:UP] == 0, np.float32(-1e9), np.float32(0.0))
            row = row + xb[bl, :UP].astype(np.float32) @ w1vec
            mkb[bl, :, :] = row.astype(ml_dtypes.bfloat16)[None, :]
        xT_t = np.ascontiguousarray(
            xTf.reshape(B_LOC, KT, P, S).transpose(0, 2, 1, 3))
        in_maps.append({
            "xT": xT_t,
            "mp": mp, "wvp": wvp,
            "bvr": bvr, "dp": dp, "mkb": mkb,
        })

    nc = build(UP)
    res = run_bass_kernel_spmd(nc, in_maps, core_ids=list(range(NCORES)))
    att = np.empty((B, S, H), dtype=np.float32)
    attw = np.zeros((B, S, S), dtype=np.float32)
    for c in range(NCORES):
        att_c = res.results[c]["att"]                    # [B_LOC, S, H] bf16, permuted rows
        awc = res.results[c]["attwc"]                    # [B_LOC, S, UP] bf16, permuted rows
        for bl in range(B_LOC):
            gb = c * B_LOC + bl
            att[gb][perms[gb]] = att_c[bl].astype(np.float32)
            tmp = np.zeros((S, S), dtype=np.float32)
            tmp[:, idxs[gb]] = awc[bl][:, :ucounts[gb]].astype(np.float32)
            attw[gb][perms[gb]] = tmp
    return att, attw
